# revision 18
# baseline (speedup 1.0000x reference)
"""Trainium2 Bass kernel for a 3-layer GIN encoder (gnn_message_passing).

Reference computation (per layer l):
    agg_i = sum_{j -> i} z_j          (scatter-add over edges)
    h     = z + agg                   (GIN eps=0, folded in as self-edges)
    z     = relu(relu(h @ w1 + b1) @ w2 + b2)

Distribution strategy (8 NeuronCores, SPMD single program):
  * Nodes block-sharded; edges partitioned by destination core so the
    aggregation is local; each layer's full activation table is AllGathered
    (the halo exchange for a dense random graph).  Internally nodes live in
    a padded index space (1280 slots/core, 30 dead) so every DMA and gather
    piece is 128-aligned; dead slots have zero adjacency everywhere.
  * Aggregation as a dense matmul with the local adjacency count matrix in
    fp8_e4m3 (counts are small ints -> exact).  The z table is also fp8
    (measured end-to-end rel err ~5e-3, bar is 2e-2), which enables
    MatmulPerfMode.DoubleRow: K=256 per instruction, 2x bf16 throughput.
  * The whole per-core adjacency (80 chunks x 1280 slots x 1B = 100KB per
    partition) stays resident in SBUF: streamed from HBM once during layer
    0, read for free in layers 1-2.
  * Each AllGather is split into one piece per MLP output group, launched
    as soon as that group's stores land; the next layer's aggregation
    consumes K-chunk pairs in piece-availability order so it starts as
    soon as the first piece arrives.
  * MLP in bf16 with hi/lo splits (3 product terms ~ fp32 accuracy),
    PSUM-accumulated; outputs transposed back via TensorE, stored fp8
    (f32 for the final layer).
"""

import os
import sys

sys.path.insert(0, "/opt/trn_rl_repo")

import numpy as np
import ml_dtypes

BF16 = ml_dtypes.bfloat16
FP8 = ml_dtypes.float8_e4m3  # TRN fp8e4 (max 240)
P = 128
NCORES = 8

# hi/lo product terms in the MLP matmuls (2 keeps rel err ~6e-3, bar 2e-2)
NSPLIT = 2
# adjacency chunks fetched per stream DMA during layer 0
ABATCH = 4

_BUILD_CACHE: dict = {}


# --------------------------------------------------------------------------
# host-side preprocessing
# --------------------------------------------------------------------------

def _config(inputs):
    x = inputs["x"]
    N, DIN = int(x.shape[0]), int(x.shape[1])
    L = 0
    while f"w1_{L}" in inputs:
        L += 1
    DH = int(inputs["w1_0"].shape[1])
    assert N % NCORES == 0
    NPC = N // NCORES              # real rows per core (1250)
    MT = (NPC + P - 1) // P        # M-tiles per core (10)
    NPC2 = MT * P                  # padded rows per core (1280)
    N2 = NCORES * NPC2             # padded node space (10240)
    KC2 = N2 // P                  # zsb chunks (80, even)
    assert DIN % P == 0 and DH % P == 0 and MT % 2 == 0
    return dict(N=N, DIN=DIN, DH=DH, L=L, NPC=NPC, MT=MT, NPC2=NPC2,
                N2=N2, KC2=KC2)


def _prep_a8(edge_index, N, NPC, NPC2, KC2):
    """Dense transposed local adjacency per core, fp8, stream-batched.

    Src/dst in the padded index space.  Returns a8[c] of shape
    [KC2//ABATCH, P, ABATCH*NPC2] fp8 with
    a8[c][b, p, j*NPC2 + s] = #edges (src_pad = (ABATCH*b+j)*128+p) ->
    (dst = c*NPC2 + s), plus one self-edge per node.
    """
    src = np.asarray(edge_index[0], dtype=np.int64)
    dst = np.asarray(edge_index[1], dtype=np.int64)
    self_ix = np.arange(N, dtype=np.int64)
    allsrc = np.concatenate([src, self_ix])
    alldst = np.concatenate([dst, self_ix])
    # real -> padded index space
    allsrc = (allsrc // NPC) * NPC2 + allsrc % NPC
    gslot = (alldst // NPC) * NPC2 + alldst % NPC

    at = np.zeros((KC2 * P, NCORES * NPC2), np.float32)
    np.add.at(at, (allsrc, gslot), 1.0)
    at8 = at.astype(FP8)

    NB = KC2 // ABATCH
    a8 = []
    for c in range(NCORES):
        sl = at8[:, c * NPC2:(c + 1) * NPC2]
        a8.append(np.ascontiguousarray(
            sl.reshape(NB, ABATCH, P, NPC2)
              .transpose(0, 2, 1, 3)
              .reshape(NB, P, ABATCH * NPC2)))
    return a8


def _prep_xz(x, DIN, NPC, NPC2, KC2):
    """x in zsb layout over the padded space: xz[p, k*DIN+f] = x_pad[k*128+p, f]."""
    xf = np.zeros((KC2 * P, DIN), np.float32)
    xv = np.asarray(x, dtype=np.float32).reshape(NCORES, NPC, DIN)
    xf.reshape(NCORES, NPC2, DIN)[:, :NPC] = xv
    xz = xf.reshape(KC2, P, DIN).transpose(1, 0, 2).reshape(P, KC2 * DIN)
    return np.ascontiguousarray(xz).astype(FP8)


# --------------------------------------------------------------------------
# bass program
# --------------------------------------------------------------------------

def _build(N, DIN, DH, L, NPC, MT, NPC2, N2, KC2):
    from concourse import bacc, mybir, tile

    f32 = mybir.dt.float32
    bf = mybir.dt.bfloat16
    f8 = mybir.dt.float8e4
    SUB = mybir.AluOpType.subtract
    RELU = mybir.ActivationFunctionType.Relu
    DR = mybir.MatmulPerfMode.DoubleRow

    NKT2 = DH // P            # K/M tiles of the hidden dim (2)
    PAIRS = KC2 // 2
    NB = KC2 // ABATCH        # layer-0 stream batches
    # dst-slot groups; aligned with MLP M-tile groups of 4 (512 rows)
    NG = [(n0, min(512, NPC2 - n0)) for n0 in range(0, NPC2, 512)]
    GROUPS = [(g0, min(4, MT - g0)) for g0 in range(0, MT, 4)]

    # gather piece (per MLP group) that provides chunk k of the z table
    def chunk_piece(k):
        kl = k % MT
        for gi, (g0, gm) in enumerate(GROUPS):
            if kl < g0 + gm:
                return gi
        return len(GROUPS) - 1
    pair_order = sorted(range(PAIRS),
                        key=lambda p: (chunk_piece(2 * p), p))

    nc = bacc.Bacc(num_devices=NCORES)

    xzin = nc.dram_tensor("xz", [P, KC2 * DIN], f8, kind="ExternalInput")
    a8in = nc.dram_tensor("a8", [NB, P, ABATCH * NPC2], f8, kind="ExternalInput")
    identbin = nc.dram_tensor("identb", [P, P], bf, kind="ExternalInput")
    identfin = nc.dram_tensor("identf", [P, P], f32, kind="ExternalInput")
    win = {}
    for l in range(L):
        din = DIN if l == 0 else DH
        for nm, shp in [
            ("w1h", [din, DH]), ("w1l", [din, DH]),
            ("w2h", [DH, DH]), ("w2l", [DH, DH]),
        ]:
            win[(nm, l)] = nc.dram_tensor(f"{nm}_{l}", shp, bf, kind="ExternalInput")
        for nm in ("b1", "b2"):
            win[(nm, l)] = nc.dram_tensor(f"{nm}_{l}", [DH, 1], f32, kind="ExternalInput")
    zout = nc.dram_tensor("zout", [NPC, DH], f32, kind="ExternalOutput")

    with tile.TileContext(nc) as tc:
        with tc.tile_pool(name="const", bufs=1) as cp, \
             tc.tile_pool(name="zsbpool", bufs=2) as zsp, \
             tc.tile_pool(name="hpool", bufs=1) as hp, \
             tc.tile_pool(name="spool", bufs=2) as sp, \
             tc.tile_pool(name="zpool", bufs=1) as zp, \
             tc.tile_pool(name="zrpool", bufs=3) as zrp, \
             tc.tile_pool(name="hpsum", bufs=1, space="PSUM") as hpsum, \
             tc.tile_pool(name="mlppsum", bufs=2, space="PSUM") as mlppool, \
             tc.tile_pool(name="drampool", bufs=1, space="DRAM") as dp:

            # ---------------- resident constants ----------------
            identb_t = cp.tile([P, P], bf, name="identb_t")
            nc.gpsimd.dma_start(out=identb_t[:], in_=identbin[:, :])
            identf_t = cp.tile([P, P], f32, name="identf_t")
            nc.gpsimd.dma_start(out=identf_t[:], in_=identfin[:, :])

            ccsrc = dp.tile([P, 4], bf, name="ccsrc")
            nc.gpsimd.dma_start(out=ccsrc[:, :], in_=identb_t[:, 0:4])
            ccwarm = dp.tile([NCORES * P, 4], bf, name="ccwarm", addr_space="Shared")
            nc.gpsimd.collective_compute(
                "AllGather",
                mybir.AluOpType.bypass,
                replica_groups=[list(range(NCORES))],
                ins=[ccsrc[:, :].opt()],
                outs=[ccwarm[:, :].opt()],
            )

            wt = {}
            for l in range(L):
                din = DIN if l == 0 else DH
                nkt = din // P
                for nm, nk in (("w1h", nkt), ("w1l", nkt),
                               ("w2h", NKT2), ("w2l", NKT2)):
                    t = cp.tile([P, nk * DH], bf, name=f"{nm}{l}_t")
                    for kt in range(nk):
                        nc.gpsimd.dma_start(
                            out=t[:, kt * DH:(kt + 1) * DH],
                            in_=win[(nm, l)][kt * P:(kt + 1) * P, :])
                    wt[(nm, l)] = t
                for nm in ("b1", "b2"):
                    t = cp.tile([P, NKT2], f32, name=f"{nm}{l}_t")
                    for mo in range(NKT2):
                        nc.gpsimd.dma_start(
                            out=t[:, mo:mo + 1],
                            in_=win[(nm, l)][mo * P:(mo + 1) * P, :])
                    wt[(nm, l)] = t

            # resident adjacency: the whole per-core A.T in fp8
            acache = cp.tile([P, KC2, NPC2], f8, name="acache")

            # layer-boundary activation tables: one shared piece per
            # (layer, MLP group); piece gi holds rows [g0*P, (g0+gm)*P) of
            # every core's padded shard, concatenated by core
            zloc = [dp.tile([NPC2, DH], f8, name=f"zloc{l}")
                    for l in range(L - 1)]
            zfp = [[dp.tile([NCORES * gm * P, DH], f8,
                            name=f"zfp{l}_{gi}", addr_space="Shared")
                    for gi, (g0, gm) in enumerate(GROUPS)]
                   for l in range(L - 1)]

            # ---------------- layers ----------------
            for l in range(L):
                din = DIN if l == 0 else DH
                nkt = din // P
                last = (l == L - 1)

                # activation table -> SBUF, node-major chunks:
                # zsb[p, k, f] = z_pad[k*128+p, f]
                zsb = zsp.tile([P, KC2, din], f8, name=f"zsb_{l}", tag="zsb")
                if l == 0:
                    nc.scalar.dma_start(
                        out=zsb[:, :, :].rearrange("p k f -> p (k f)"),
                        in_=xzin[:, :])
                else:
                    # per (piece, core) aligned loads, piece-availability
                    # order, alternating issue queues
                    for gi, (g0, gm) in enumerate(GROUPS):
                        for q in range(NCORES):
                            k0 = q * MT + g0
                            eng = nc.scalar if q % 2 == 0 else nc.sync
                            eng.dma_start(
                                out=zsb[:, k0:k0 + gm, :],
                                in_=zfp[l - 1][gi]
                                    [q * gm * P:(q + 1) * gm * P, :]
                                    .rearrange("(k p) f -> p k f", p=P))

                # --- aggregation: h.T = z.T @ Aloc.T, fp8 DoubleRow pairs,
                # k-outer so each stationary zsb slice is loaded once
                hps = [hpsum.tile([P, nkt * 512], f32,
                                  name=f"hps{gi}_{l}", tag=f"hps{gi}")
                       for gi in range(len(NG))]
                porder = list(range(PAIRS)) if l == 0 else pair_order

                def agg_phase(gis, stream, pr0=0, pr1=PAIRS):
                    for pi, p in enumerate(porder[pr0:pr1], start=pr0):
                        if stream and p % (ABATCH // 2) == 0:
                            b = p // (ABATCH // 2)
                            nc.sync.dma_start(
                                out=acache[:, b * ABATCH:(b + 1) * ABATCH, :]
                                    .rearrange("p k s -> p (k s)"),
                                in_=a8in[b, :, :])
                        for mf in range(nkt):
                            for gi in gis:
                                n0, nn = NG[gi]
                                nc.tensor.matmul(
                                    out=hps[gi][:, mf * 512: mf * 512 + nn],
                                    lhsT=zsb[:, 2 * p:2 * p + 2,
                                             mf * P:(mf + 1) * P],
                                    rhs=acache[:, 2 * p:2 * p + 2, n0:n0 + nn],
                                    start=(pi == 0),
                                    stop=(pi == PAIRS - 1),
                                    perf_mode=DR,
                                )

                if l == 0:
                    agg_phase([0, 1, 2], stream=True)
                else:
                    agg_phase([0], stream=False)

                # --- MLP per dst group (aligned with NG: 512 rows each)
                hhi = [hp.tile([P, NPC2], bf, name=f"hhi{mf}_{l}", tag=f"hhi{mf}")
                       for mf in range(nkt)]
                hlo = [hp.tile([P, NPC2], bf, name=f"hlo{mf}_{l}", tag=f"hlo{mf}")
                       for mf in range(nkt)]
                zT = [zp.tile([P, NPC2], f32 if last else bf,
                              name=f"zT{mo}_{l}",
                              tag=f"zT{mo}{'f' if last else 'b'}")
                      for mo in range(NKT2)]
                s1buf = {}

                def mlp_in(gi):
                    g0, gm = GROUPS[gi]
                    rows = gm * P
                    r0 = g0 * P
                    # split this group's h into hi/lo bf16
                    for mf in range(nkt):
                        nc.vector.tensor_copy(
                            out=hhi[mf][:, r0:r0 + rows],
                            in_=hps[gi][:, mf * 512: mf * 512 + rows])
                        nc.vector.tensor_tensor(
                            out=hlo[mf][:, r0:r0 + rows],
                            in0=hps[gi][:, mf * 512: mf * 512 + rows],
                            in1=hhi[mf][:, r0:r0 + rows],
                            op=SUB)

                    combos1 = [("w1h", hhi), ("w1h", hlo), ("w1l", hhi)][:NSPLIT]
                    s1h, s1l = [], []
                    for mo in range(NKT2):
                        p1 = mlppool.tile([P, 512], f32,
                                          name=f"p1_{l}_{g0}_{mo}", tag="mlp")
                        tot = len(combos1) * nkt
                        step = 0
                        for (wn, ht) in combos1:
                            for kt in range(nkt):
                                nc.tensor.matmul(
                                    out=p1[:, :rows],
                                    lhsT=wt[(wn, l)][:, kt * DH + mo * P: kt * DH + (mo + 1) * P],
                                    rhs=ht[kt][:, r0:r0 + rows],
                                    start=(step == 0), stop=(step == tot - 1))
                                step += 1
                        s1f = sp.tile([P, 512], f32, name=f"s1f_{l}_{g0}_{mo}", tag="s1f")
                        nc.scalar.activation(
                            out=s1f[:, :rows], in_=p1[:, :rows], func=RELU,
                            bias=wt[("b1", l)][:, mo:mo + 1])
                        sh = sp.tile([P, 512], bf, name=f"s1h_{l}_{g0}_{mo}", tag=f"s1h{mo}")
                        nc.vector.tensor_copy(out=sh[:, :rows], in_=s1f[:, :rows])
                        sl = sp.tile([P, 512], bf, name=f"s1l_{l}_{g0}_{mo}", tag=f"s1l{mo}")
                        nc.vector.tensor_tensor(
                            out=sl[:, :rows], in0=s1f[:, :rows], in1=sh[:, :rows], op=SUB)
                        s1h.append(sh)
                        s1l.append(sl)
                    s1buf[gi] = (s1h, s1l)

                def mlp_out(gi):
                    g0, gm = GROUPS[gi]
                    rows = gm * P
                    r0 = g0 * P
                    s1h, s1l = s1buf.pop(gi)
                    combos2 = [("w2h", s1h), ("w2h", s1l), ("w2l", s1h)][:NSPLIT]
                    for mo in range(NKT2):
                        p2 = mlppool.tile([P, 512], f32,
                                          name=f"p2_{l}_{g0}_{mo}", tag="mlp")
                        tot = len(combos2) * NKT2
                        step = 0
                        for (wn, st) in combos2:
                            for kt in range(NKT2):
                                nc.tensor.matmul(
                                    out=p2[:, :rows],
                                    lhsT=wt[(wn, l)][:, kt * DH + mo * P: kt * DH + (mo + 1) * P],
                                    rhs=st[kt][:, :rows],
                                    start=(step == 0), stop=(step == tot - 1))
                                step += 1
                        nc.scalar.activation(
                            out=zT[mo][:, r0:r0 + rows], in_=p2[:, :rows], func=RELU,
                            bias=wt[("b2", l)][:, mo:mo + 1])

                    # transpose back to row-major + store
                    ident = identf_t if last else identb_t
                    for m in range(g0, g0 + gm):
                        rows_m = min(P, NPC - m * P) if last else P
                        tp = mlppool.tile([P, NKT2 * P], f32 if last else bf,
                                          name=f"tp_{l}_{m}", tag="mlp")
                        for mo in range(NKT2):
                            nc.tensor.transpose(
                                out=tp[:, mo * P:(mo + 1) * P],
                                in_=zT[mo][:, m * P:(m + 1) * P],
                                identity=ident[:])
                        zr = zrp.tile([P, NKT2 * P], f32 if last else f8,
                                      name=f"zr_{l}_{m}", tag="zr")
                        nc.vector.tensor_copy(out=zr[:], in_=tp[:])
                        dst = zout if last else zloc[l]
                        nc.sync.dma_start(
                            out=dst[m * P: m * P + rows_m, :],
                            in_=zr[:rows_m, :])

                    # launch this group's gather piece as soon as its
                    # stores land; the next layer starts on piece 0
                    if not last:
                        nc.gpsimd.collective_compute(
                            "AllGather",
                            mybir.AluOpType.bypass,
                            replica_groups=[list(range(NCORES))],
                            ins=[zloc[l][r0:r0 + rows, :].opt()],
                            outs=[zfp[l][gi][:, :].opt()],
                        )

                if l == 0:
                    mlp_in(0)
                    mlp_out(0)
                else:
                    agg_phase([1, 2], stream=False, pr0=0, pr1=2)
                    mlp_in(0)
                    agg_phase([1, 2], stream=False, pr0=2, pr1=6)
                    mlp_out(0)
                    agg_phase([1, 2], stream=False, pr0=6)
                mlp_in(1)
                mlp_in(2)
                mlp_out(1)
                mlp_out(2)

    nc.compile()
    return nc


# --------------------------------------------------------------------------
# entry point
# --------------------------------------------------------------------------

def _make_in_maps(inputs, cfg, a8):
    DIN, DH, L = cfg["DIN"], cfg["DH"], cfg["L"]
    xz = _prep_xz(inputs["x"], DIN, cfg["NPC"], cfg["NPC2"], cfg["KC2"])
    identb = np.eye(P, dtype=np.float32).astype(BF16)
    identf = np.eye(P, dtype=np.float32)

    shared = {"xz": xz, "identb": identb, "identf": identf}
    for l in range(L):
        w1 = np.asarray(inputs[f"w1_{l}"], dtype=np.float32)
        w2 = np.asarray(inputs[f"w2_{l}"], dtype=np.float32)
        w1h = w1.astype(BF16)
        w2h = w2.astype(BF16)
        shared[f"w1h_{l}"] = w1h
        shared[f"w1l_{l}"] = (w1 - w1h.astype(np.float32)).astype(BF16)
        shared[f"w2h_{l}"] = w2h
        shared[f"w2l_{l}"] = (w2 - w2h.astype(np.float32)).astype(BF16)
        shared[f"b1_{l}"] = np.asarray(
            inputs[f"b1_{l}"], dtype=np.float32).reshape(DH, 1)
        shared[f"b2_{l}"] = np.asarray(
            inputs[f"b2_{l}"], dtype=np.float32).reshape(DH, 1)

    in_maps = []
    for c in range(NCORES):
        m = dict(shared)
        m["a8"] = a8[c]
        in_maps.append(m)
    return in_maps


def get_program(inputs):
    """Build (or fetch cached) the bass program + per-core input maps."""
    cfg = _config(inputs)
    a8 = _prep_a8(inputs["edge_index"], cfg["N"], cfg["NPC"], cfg["NPC2"],
                  cfg["KC2"])
    key = (cfg["N"], cfg["DIN"], cfg["DH"], cfg["L"], NSPLIT)
    if key not in _BUILD_CACHE:
        _BUILD_CACHE[key] = _build(
            cfg["N"], cfg["DIN"], cfg["DH"], cfg["L"],
            cfg["NPC"], cfg["MT"], cfg["NPC2"], cfg["N2"], cfg["KC2"])
    nc = _BUILD_CACHE[key]
    in_maps = _make_in_maps(inputs, cfg, a8)
    return nc, in_maps, cfg


def kernel(**inputs):
    nc, in_maps, cfg = get_program(inputs)

    if os.environ.get("KERNEL_USE_SIM"):
        from concourse.bass_interp import MultiCoreSim
        sim = MultiCoreSim(nc, num_cores=NCORES)
        cores = list(sim.cores.values())
        for cid, cs in enumerate(cores):
            for name, val in in_maps[cid].items():
                cs.tensor(name)[:] = val
        sim.simulate(check_with_hw=False)
        parts = [np.asarray(cs.tensor("zout")) for cs in cores]
    else:
        from concourse import bass_utils
        res = bass_utils.run_bass_kernel_spmd(
            nc, in_maps, core_ids=list(range(NCORES)),
            trace=bool(os.environ.get("KERNEL_TRACE")),
        )
        kernel.last_results = res
        parts = [res.results[c]["zout"] for c in range(NCORES)]

    out = np.concatenate(parts, axis=0).astype(np.float32)
    return out


# revision 19
# speedup vs baseline: 1.0835x; 1.0835x over previous
"""Trainium2 Bass kernel for a 3-layer GIN encoder (gnn_message_passing).

Reference computation (per layer l):
    agg_i = sum_{j -> i} z_j          (scatter-add over edges)
    h     = z + agg                   (GIN eps=0, folded in as self-edges)
    z     = relu(relu(h @ w1 + b1) @ w2 + b2)

Distribution strategy (8 NeuronCores, SPMD single program):
  * Nodes block-sharded; edges partitioned by destination core so the
    aggregation is local; each layer's full activation table is AllGathered
    (the halo exchange for a dense random graph).  Internally nodes live in
    a padded index space (1280 slots/core, 30 dead) so every DMA and gather
    piece is 128-aligned; dead slots have zero adjacency everywhere.
  * Aggregation as a dense matmul with the local adjacency count matrix in
    fp8_e4m3 (counts are small ints -> exact).  The z table is also fp8
    (measured end-to-end rel err ~5e-3, bar is 2e-2), which enables
    MatmulPerfMode.DoubleRow: K=256 per instruction, 2x bf16 throughput.
  * The whole per-core adjacency (80 chunks x 1280 slots x 1B = 100KB per
    partition) stays resident in SBUF: streamed from HBM once during layer
    0, read for free in layers 1-2.
  * Each AllGather is split into one piece per MLP output group, launched
    as soon as that group's stores land; the next layer's aggregation
    consumes K-chunk pairs in piece-availability order so it starts as
    soon as the first piece arrives.
  * MLP in bf16 with hi/lo splits (3 product terms ~ fp32 accuracy),
    PSUM-accumulated; outputs transposed back via TensorE, stored fp8
    (f32 for the final layer).
"""

import os
import sys

sys.path.insert(0, "/opt/trn_rl_repo")

import numpy as np
import ml_dtypes

BF16 = ml_dtypes.bfloat16
FP8 = ml_dtypes.float8_e4m3  # TRN fp8e4 (max 240)
P = 128
NCORES = 8

# hi/lo product terms in the MLP matmuls (2 keeps rel err ~6e-3, bar 2e-2)
NSPLIT = 2
# adjacency chunks fetched per stream DMA during layer 0
ABATCH = 4

_BUILD_CACHE: dict = {}


# --------------------------------------------------------------------------
# host-side preprocessing
# --------------------------------------------------------------------------

def _config(inputs):
    x = inputs["x"]
    N, DIN = int(x.shape[0]), int(x.shape[1])
    L = 0
    while f"w1_{L}" in inputs:
        L += 1
    DH = int(inputs["w1_0"].shape[1])
    assert N % NCORES == 0
    NPC = N // NCORES              # real rows per core (1250)
    MT = (NPC + P - 1) // P        # M-tiles per core (10)
    NPC2 = MT * P                  # padded rows per core (1280)
    N2 = NCORES * NPC2             # padded node space (10240)
    KC2 = N2 // P                  # zsb chunks (80, even)
    assert DIN % P == 0 and DH % P == 0 and MT % 2 == 0
    return dict(N=N, DIN=DIN, DH=DH, L=L, NPC=NPC, MT=MT, NPC2=NPC2,
                N2=N2, KC2=KC2)


def _prep_a8(edge_index, N, NPC, NPC2, KC2):
    """Dense transposed local adjacency per core, fp8, stream-batched.

    Src/dst in the padded index space.  Returns a8[c] of shape
    [KC2//ABATCH, P, ABATCH*NPC2] fp8 with
    a8[c][b, p, j*NPC2 + s] = #edges (src_pad = (ABATCH*b+j)*128+p) ->
    (dst = c*NPC2 + s), plus one self-edge per node.
    """
    src = np.asarray(edge_index[0], dtype=np.int64)
    dst = np.asarray(edge_index[1], dtype=np.int64)
    self_ix = np.arange(N, dtype=np.int64)
    allsrc = np.concatenate([src, self_ix])
    alldst = np.concatenate([dst, self_ix])
    # real -> padded index space
    allsrc = (allsrc // NPC) * NPC2 + allsrc % NPC
    gslot = (alldst // NPC) * NPC2 + alldst % NPC

    at = np.zeros((KC2 * P, NCORES * NPC2), np.float32)
    np.add.at(at, (allsrc, gslot), 1.0)
    at8 = at.astype(FP8)

    NB = KC2 // ABATCH
    a8 = []
    for c in range(NCORES):
        sl = at8[:, c * NPC2:(c + 1) * NPC2]
        a8.append(np.ascontiguousarray(
            sl.reshape(NB, ABATCH, P, NPC2)
              .transpose(0, 2, 1, 3)
              .reshape(NB, P, ABATCH * NPC2)))
    return a8


def _prep_xz(x, DIN, NPC, NPC2, KC2):
    """x in zsb layout over the padded space: xz[p, k*DIN+f] = x_pad[k*128+p, f]."""
    xf = np.zeros((KC2 * P, DIN), np.float32)
    xv = np.asarray(x, dtype=np.float32).reshape(NCORES, NPC, DIN)
    xf.reshape(NCORES, NPC2, DIN)[:, :NPC] = xv
    xz = xf.reshape(KC2, P, DIN).transpose(1, 0, 2).reshape(P, KC2 * DIN)
    return np.ascontiguousarray(xz).astype(FP8)


# --------------------------------------------------------------------------
# bass program
# --------------------------------------------------------------------------

def _build(N, DIN, DH, L, NPC, MT, NPC2, N2, KC2):
    from concourse import bacc, mybir, tile

    f32 = mybir.dt.float32
    bf = mybir.dt.bfloat16
    f8 = mybir.dt.float8e4
    SUB = mybir.AluOpType.subtract
    RELU = mybir.ActivationFunctionType.Relu
    DR = mybir.MatmulPerfMode.DoubleRow

    NKT2 = DH // P            # K/M tiles of the hidden dim (2)
    PAIRS = KC2 // 2
    NB = KC2 // ABATCH        # layer-0 stream batches
    # dst-slot groups; aligned with MLP M-tile groups of 4 (512 rows)
    NG = [(n0, min(512, NPC2 - n0)) for n0 in range(0, NPC2, 512)]
    GROUPS = [(g0, min(4, MT - g0)) for g0 in range(0, MT, 4)]

    # gather piece (per MLP group) that provides chunk k of the z table
    def chunk_piece(k):
        kl = k % MT
        for gi, (g0, gm) in enumerate(GROUPS):
            if kl < g0 + gm:
                return gi
        return len(GROUPS) - 1
    pair_order = sorted(range(PAIRS),
                        key=lambda p: (chunk_piece(2 * p), p))

    nc = bacc.Bacc(num_devices=NCORES)

    xzin = nc.dram_tensor("xz", [P, KC2 * DIN], f8, kind="ExternalInput")
    a8in = nc.dram_tensor("a8", [NB, P, ABATCH * NPC2], f8, kind="ExternalInput")
    identbin = nc.dram_tensor("identb", [P, P], bf, kind="ExternalInput")
    identfin = nc.dram_tensor("identf", [P, P], f32, kind="ExternalInput")
    win = {}
    for l in range(L):
        din = DIN if l == 0 else DH
        for nm, shp in [
            ("w1h", [din, DH]), ("w1l", [din, DH]),
            ("w2h", [DH, DH]), ("w2l", [DH, DH]),
        ]:
            win[(nm, l)] = nc.dram_tensor(f"{nm}_{l}", shp, bf, kind="ExternalInput")
        for nm in ("b1", "b2"):
            win[(nm, l)] = nc.dram_tensor(f"{nm}_{l}", [DH, 1], f32, kind="ExternalInput")
    zout = nc.dram_tensor("zout", [NPC, DH], f32, kind="ExternalOutput")

    with tile.TileContext(nc) as tc:
        with tc.tile_pool(name="const", bufs=1) as cp, \
             tc.tile_pool(name="zsbpool", bufs=1) as zsp, \
             tc.tile_pool(name="hpool", bufs=1) as hp, \
             tc.tile_pool(name="spool", bufs=2) as sp, \
             tc.tile_pool(name="zpool", bufs=1) as zp, \
             tc.tile_pool(name="zrpool", bufs=3) as zrp, \
             tc.tile_pool(name="hpsum", bufs=1, space="PSUM") as hpsum, \
             tc.tile_pool(name="mlppsum", bufs=2, space="PSUM") as mlppool, \
             tc.tile_pool(name="drampool", bufs=1, space="DRAM") as dp:

            # ---------------- resident constants ----------------
            identb_t = cp.tile([P, P], bf, name="identb_t")
            nc.gpsimd.dma_start(out=identb_t[:], in_=identbin[:, :])
            identf_t = cp.tile([P, P], f32, name="identf_t")
            nc.gpsimd.dma_start(out=identf_t[:], in_=identfin[:, :])

            ccsrc = dp.tile([P, 4], bf, name="ccsrc")
            nc.gpsimd.dma_start(out=ccsrc[:, :], in_=identb_t[:, 0:4])
            ccwarm = dp.tile([NCORES * P, 4], bf, name="ccwarm", addr_space="Shared")
            nc.gpsimd.collective_compute(
                "AllGather",
                mybir.AluOpType.bypass,
                replica_groups=[list(range(NCORES))],
                ins=[ccsrc[:, :].opt()],
                outs=[ccwarm[:, :].opt()],
            )

            wt = {}
            for l in range(L):
                din = DIN if l == 0 else DH
                nkt = din // P
                for nm, nk in (("w1h", nkt), ("w1l", nkt),
                               ("w2h", NKT2), ("w2l", NKT2)):
                    t = cp.tile([P, nk * DH], bf, name=f"{nm}{l}_t")
                    for kt in range(nk):
                        nc.gpsimd.dma_start(
                            out=t[:, kt * DH:(kt + 1) * DH],
                            in_=win[(nm, l)][kt * P:(kt + 1) * P, :])
                    wt[(nm, l)] = t
                for nm in ("b1", "b2"):
                    t = cp.tile([P, NKT2], f32, name=f"{nm}{l}_t")
                    for mo in range(NKT2):
                        nc.gpsimd.dma_start(
                            out=t[:, mo:mo + 1],
                            in_=win[(nm, l)][mo * P:(mo + 1) * P, :])
                    wt[(nm, l)] = t

            # resident adjacency: the whole per-core A.T in fp8
            acache = cp.tile([P, KC2, NPC2], f8, name="acache")

            # layer-boundary activation tables: one shared piece per
            # (layer, MLP group); piece gi holds rows [g0*P, (g0+gm)*P) of
            # every core's padded shard, concatenated by core
            zloc = [dp.tile([NPC2, DH], f8, name=f"zloc{l}")
                    for l in range(L - 1)]
            zfp = [[dp.tile([NCORES * gm * P, DH], f8,
                            name=f"zfp{l}_{gi}", addr_space="Shared")
                    for gi, (g0, gm) in enumerate(GROUPS)]
                   for l in range(L - 1)]

            # ---------------- layers ----------------
            for l in range(L):
                din = DIN if l == 0 else DH
                nkt = din // P
                last = (l == L - 1)

                # activation table -> SBUF, node-major chunks:
                # zsb[p, k, f] = z_pad[k*128+p, f]
                zsb = zsp.tile([P, KC2, din], f8, name=f"zsb_{l}", tag="zsb")
                if l == 0:
                    nc.scalar.dma_start(
                        out=zsb[:, :, :].rearrange("p k f -> p (k f)"),
                        in_=xzin[:, :])
                else:
                    # per (piece, core) aligned loads, piece-availability
                    # order, alternating issue queues
                    for gi, (g0, gm) in enumerate(GROUPS):
                        for q in range(NCORES):
                            k0 = q * MT + g0
                            eng = nc.scalar if q % 2 == 0 else nc.sync
                            eng.dma_start(
                                out=zsb[:, k0:k0 + gm, :],
                                in_=zfp[l - 1][gi]
                                    [q * gm * P:(q + 1) * gm * P, :]
                                    .rearrange("(k p) f -> p k f", p=P))

                # --- aggregation: h.T = z.T @ Aloc.T, fp8 DoubleRow pairs,
                # k-outer so each stationary zsb slice is loaded once
                hps = [hpsum.tile([P, nkt * 512], f32,
                                  name=f"hps{gi}_{l}", tag=f"hps{gi}")
                       for gi in range(len(NG))]
                porder = list(range(PAIRS)) if l == 0 else pair_order

                def agg_phase(gis, stream, pr0=0, pr1=PAIRS):
                    for pi, p in enumerate(porder[pr0:pr1], start=pr0):
                        if stream and p % (ABATCH // 2) == 0:
                            b = p // (ABATCH // 2)
                            nc.sync.dma_start(
                                out=acache[:, b * ABATCH:(b + 1) * ABATCH, :]
                                    .rearrange("p k s -> p (k s)"),
                                in_=a8in[b, :, :])
                        for mf in range(nkt):
                            for gi in gis:
                                n0, nn = NG[gi]
                                nc.tensor.matmul(
                                    out=hps[gi][:, mf * 512: mf * 512 + nn],
                                    lhsT=zsb[:, 2 * p:2 * p + 2,
                                             mf * P:(mf + 1) * P],
                                    rhs=acache[:, 2 * p:2 * p + 2, n0:n0 + nn],
                                    start=(pi == 0),
                                    stop=(pi == PAIRS - 1),
                                    perf_mode=DR,
                                )

                if l == 0:
                    agg_phase([0, 1, 2], stream=True)
                else:
                    agg_phase([0], stream=False)

                # --- MLP per dst group (aligned with NG: 512 rows each)
                hhi = [hp.tile([P, NPC2], bf, name=f"hhi{mf}_{l}", tag=f"hhi{mf}")
                       for mf in range(nkt)]
                hlo = [hp.tile([P, NPC2], bf, name=f"hlo{mf}_{l}", tag=f"hlo{mf}")
                       for mf in range(nkt)]
                zT = [zp.tile([P, NPC2], f32 if last else bf,
                              name=f"zT{mo}_{l}",
                              tag=f"zT{mo}{'f' if last else 'b'}")
                      for mo in range(NKT2)]
                s1buf = {}

                def mlp_in(gi):
                    g0, gm = GROUPS[gi]
                    rows = gm * P
                    r0 = g0 * P
                    # split this group's h into hi/lo bf16
                    for mf in range(nkt):
                        nc.vector.tensor_copy(
                            out=hhi[mf][:, r0:r0 + rows],
                            in_=hps[gi][:, mf * 512: mf * 512 + rows])
                        nc.vector.tensor_tensor(
                            out=hlo[mf][:, r0:r0 + rows],
                            in0=hps[gi][:, mf * 512: mf * 512 + rows],
                            in1=hhi[mf][:, r0:r0 + rows],
                            op=SUB)

                    combos1 = [("w1h", hhi), ("w1h", hlo), ("w1l", hhi)][:NSPLIT]
                    s1h, s1l = [], []
                    for mo in range(NKT2):
                        p1 = mlppool.tile([P, 512], f32,
                                          name=f"p1_{l}_{g0}_{mo}", tag="mlp")
                        tot = len(combos1) * nkt
                        step = 0
                        for (wn, ht) in combos1:
                            for kt in range(nkt):
                                nc.tensor.matmul(
                                    out=p1[:, :rows],
                                    lhsT=wt[(wn, l)][:, kt * DH + mo * P: kt * DH + (mo + 1) * P],
                                    rhs=ht[kt][:, r0:r0 + rows],
                                    start=(step == 0), stop=(step == tot - 1))
                                step += 1
                        s1f = sp.tile([P, 512], f32, name=f"s1f_{l}_{g0}_{mo}", tag="s1f")
                        nc.scalar.activation(
                            out=s1f[:, :rows], in_=p1[:, :rows], func=RELU,
                            bias=wt[("b1", l)][:, mo:mo + 1])
                        sh = sp.tile([P, 512], bf, name=f"s1h_{l}_{g0}_{mo}", tag=f"s1h{mo}")
                        nc.vector.tensor_copy(out=sh[:, :rows], in_=s1f[:, :rows])
                        sl = sp.tile([P, 512], bf, name=f"s1l_{l}_{g0}_{mo}", tag=f"s1l{mo}")
                        nc.vector.tensor_tensor(
                            out=sl[:, :rows], in0=s1f[:, :rows], in1=sh[:, :rows], op=SUB)
                        s1h.append(sh)
                        s1l.append(sl)
                    s1buf[gi] = (s1h, s1l)

                def mlp_out(gi):
                    g0, gm = GROUPS[gi]
                    rows = gm * P
                    r0 = g0 * P
                    s1h, s1l = s1buf.pop(gi)
                    combos2 = [("w2h", s1h), ("w2h", s1l), ("w2l", s1h)][:NSPLIT]
                    for mo in range(NKT2):
                        p2 = mlppool.tile([P, 512], f32,
                                          name=f"p2_{l}_{g0}_{mo}", tag="mlp")
                        tot = len(combos2) * NKT2
                        step = 0
                        for (wn, st) in combos2:
                            for kt in range(NKT2):
                                nc.tensor.matmul(
                                    out=p2[:, :rows],
                                    lhsT=wt[(wn, l)][:, kt * DH + mo * P: kt * DH + (mo + 1) * P],
                                    rhs=st[kt][:, :rows],
                                    start=(step == 0), stop=(step == tot - 1))
                                step += 1
                        nc.scalar.activation(
                            out=zT[mo][:, r0:r0 + rows], in_=p2[:, :rows], func=RELU,
                            bias=wt[("b2", l)][:, mo:mo + 1])

                    # transpose back to row-major + store
                    ident = identf_t if last else identb_t
                    for m in range(g0, g0 + gm):
                        rows_m = min(P, NPC - m * P) if last else P
                        tp = mlppool.tile([P, NKT2 * P], f32 if last else bf,
                                          name=f"tp_{l}_{m}", tag="mlp")
                        for mo in range(NKT2):
                            nc.tensor.transpose(
                                out=tp[:, mo * P:(mo + 1) * P],
                                in_=zT[mo][:, m * P:(m + 1) * P],
                                identity=ident[:])
                        zr = zrp.tile([P, NKT2 * P], f32 if last else f8,
                                      name=f"zr_{l}_{m}", tag="zr")
                        nc.vector.tensor_copy(out=zr[:], in_=tp[:])
                        dst = zout if last else zloc[l]
                        nc.sync.dma_start(
                            out=dst[m * P: m * P + rows_m, :],
                            in_=zr[:rows_m, :])

                    # launch this group's gather piece as soon as its
                    # stores land; the next layer starts on piece 0
                    if not last:
                        nc.gpsimd.collective_compute(
                            "AllGather",
                            mybir.AluOpType.bypass,
                            replica_groups=[list(range(NCORES))],
                            ins=[zloc[l][r0:r0 + rows, :].opt()],
                            outs=[zfp[l][gi][:, :].opt()],
                        )

                if l == 0:
                    mlp_in(0)
                    mlp_out(0)
                else:
                    mlp_in(0)
                    agg_phase([1, 2], stream=False, pr0=0, pr1=4)
                    mlp_out(0)
                    agg_phase([1, 2], stream=False, pr0=4)
                mlp_in(1)
                mlp_in(2)
                mlp_out(1)
                mlp_out(2)

    nc.compile()
    return nc


# --------------------------------------------------------------------------
# entry point
# --------------------------------------------------------------------------

def _make_in_maps(inputs, cfg, a8):
    DIN, DH, L = cfg["DIN"], cfg["DH"], cfg["L"]
    xz = _prep_xz(inputs["x"], DIN, cfg["NPC"], cfg["NPC2"], cfg["KC2"])
    identb = np.eye(P, dtype=np.float32).astype(BF16)
    identf = np.eye(P, dtype=np.float32)

    shared = {"xz": xz, "identb": identb, "identf": identf}
    for l in range(L):
        w1 = np.asarray(inputs[f"w1_{l}"], dtype=np.float32)
        w2 = np.asarray(inputs[f"w2_{l}"], dtype=np.float32)
        w1h = w1.astype(BF16)
        w2h = w2.astype(BF16)
        shared[f"w1h_{l}"] = w1h
        shared[f"w1l_{l}"] = (w1 - w1h.astype(np.float32)).astype(BF16)
        shared[f"w2h_{l}"] = w2h
        shared[f"w2l_{l}"] = (w2 - w2h.astype(np.float32)).astype(BF16)
        shared[f"b1_{l}"] = np.asarray(
            inputs[f"b1_{l}"], dtype=np.float32).reshape(DH, 1)
        shared[f"b2_{l}"] = np.asarray(
            inputs[f"b2_{l}"], dtype=np.float32).reshape(DH, 1)

    in_maps = []
    for c in range(NCORES):
        m = dict(shared)
        m["a8"] = a8[c]
        in_maps.append(m)
    return in_maps


def get_program(inputs):
    """Build (or fetch cached) the bass program + per-core input maps."""
    cfg = _config(inputs)
    a8 = _prep_a8(inputs["edge_index"], cfg["N"], cfg["NPC"], cfg["NPC2"],
                  cfg["KC2"])
    key = (cfg["N"], cfg["DIN"], cfg["DH"], cfg["L"], NSPLIT)
    if key not in _BUILD_CACHE:
        _BUILD_CACHE[key] = _build(
            cfg["N"], cfg["DIN"], cfg["DH"], cfg["L"],
            cfg["NPC"], cfg["MT"], cfg["NPC2"], cfg["N2"], cfg["KC2"])
    nc = _BUILD_CACHE[key]
    in_maps = _make_in_maps(inputs, cfg, a8)
    return nc, in_maps, cfg


def kernel(**inputs):
    nc, in_maps, cfg = get_program(inputs)

    if os.environ.get("KERNEL_USE_SIM"):
        from concourse.bass_interp import MultiCoreSim
        sim = MultiCoreSim(nc, num_cores=NCORES)
        cores = list(sim.cores.values())
        for cid, cs in enumerate(cores):
            for name, val in in_maps[cid].items():
                cs.tensor(name)[:] = val
        sim.simulate(check_with_hw=False)
        parts = [np.asarray(cs.tensor("zout")) for cs in cores]
    else:
        from concourse import bass_utils
        res = bass_utils.run_bass_kernel_spmd(
            nc, in_maps, core_ids=list(range(NCORES)),
            trace=bool(os.environ.get("KERNEL_TRACE")),
        )
        kernel.last_results = res
        parts = [res.results[c]["zout"] for c in range(NCORES)]

    out = np.concatenate(parts, axis=0).astype(np.float32)
    return out
